# revision 1
# baseline (speedup 1.0000x reference)
"""Tensor-parallel GQA attention forward for Trainium2 (8 NeuronCores).

Sharding: tensor-parallel over heads.  Each core owns 4 q-heads and 1
kv-head (wq/wk/wv output-dim shard, wo input-dim shard), processes the
full 4096-token stream, and a ReduceScatter combines the o-proj partial
sums so core c ends with output token rows [c*512, (c+1)*512).

Device pipeline per core (all matmuls float32r, ~1.6e-4 relative):
  1. QKV projections from host-pretransposed x^T, fused RoPE (even/odd
     dims pre-separated by a host permutation of wq/wk rows so the
     rotation acts on contiguous 64-wide halves), PE transposes to get
     Q^T (spilled to DRAM) and K^T (SBUF-resident).  V stays natural.
  2. Streaming attention per (batch, head, 512-token piece): S^T tile =
     K^T-chunk.T @ Q^T-piece, exp on ScalarE (no max subtraction - the
     unmasked scores are O(10)), PV and ones-row sums accumulate in
     PSUM over the 16 s-chunks, reciprocal + PE-broadcast normalize.
  3. o-proj from SBUF-resident ctx^T with host-pretransposed wo shard.
  4. ReduceScatter over the 8 cores; host concatenates the slices.
"""
import math
import numpy as np

import concourse.bacc as bacc
import concourse.mybir as mybir
import concourse.tile as tile
from concourse import bass_utils

F32R = mybir.dt.float32r
F32 = mybir.dt.float32
AF = mybir.ActivationFunctionType

N_CORES = 8
B, T, DIM = 2, 2048, 4096
N_HEADS, N_KV_HEADS, HD = 32, 8, 128
HL = N_HEADS // N_CORES            # 4 q heads per core
TOK = B * T                        # 4096
KCH = DIM // 128                   # 32 contraction chunks
NTT = TOK // 128                   # 32 token tiles
QW = HL * HD                       # 512
PROJW = QW + 2 * HD                # 768 (q | k | v)
SCALE = 1.0 / math.sqrt(HD)
NSC = T // 128                     # 16 s-chunks per batch
NTP = T // 512                     # 4 t-pieces per batch
OSL = TOK // N_CORES               # 512 output rows per core

_CACHE = {}


def _build(collective=True, reps=1):
    nc = bacc.Bacc("TRN2", target_bir_lowering=False, debug=False,
                   num_devices=N_CORES if collective else 1)
    xT = nc.dram_tensor("xT", [DIM, TOK], F32R, kind="ExternalInput")
    wqkvT = nc.dram_tensor("wqkvT", [DIM, PROJW], F32R, kind="ExternalInput")
    woA = nc.dram_tensor("woA", [DIM, DIM], F32R, kind="ExternalInput")
    cosn = nc.dram_tensor("cosn", [TOK, 4 * 64], F32R, kind="ExternalInput")
    sinn = nc.dram_tensor("sinn", [TOK, 4 * 64], F32R, kind="ExternalInput")
    ones_col = nc.dram_tensor("ones_col", [128, 1], F32R, kind="ExternalInput")
    ones_row = nc.dram_tensor("ones_row", [1, 128], F32R, kind="ExternalInput")
    ident = nc.dram_tensor("ident", [128, 128], F32R, kind="ExternalInput")
    out_sl = nc.dram_tensor("out_sl", [DIM, OSL], F32, kind="ExternalOutput")

    with tile.TileContext(nc) as tc:
        with (
            nc.allow_low_precision(reason="float32r intermediates are f32 bits"),
            tc.tile_pool(name="res", bufs=1) as res,
            tc.tile_pool(name="dram", bufs=1, space="DRAM") as dram,
        ):
            kT_all = res.tile([128, TOK], F32R, tag="kT")
            v_all = res.tile([128, TOK], F32R, tag="v")
            oc_t = res.tile([128, 1], F32R, tag="oc")
            or_t = res.tile([1, 128], F32R, tag="or")
            id_t = res.tile([128, 128], F32R, tag="id")
            nc.sync.dma_start(out=oc_t[:], in_=ones_col[:])
            nc.sync.dma_start(out=or_t[:], in_=ones_row[:])
            nc.sync.dma_start(out=id_t[:], in_=ident[:])

            qT_d = dram.tile([QW, TOK], F32R)
            a2a_in = dram.tile([TOK, 512], F32R)   # [group j][c-local][t]
            a2a_out = dram.tile([TOK, 512], F32R)  # [core i][c_i][my t]

            for _rep in range(reps):
                # ------------- Phase 1: projections + RoPE + transposes ------
                with (
                    tc.tile_pool(name="p1w", bufs=1) as p1w,
                    tc.tile_pool(name="p1s", bufs=2) as p1s,
                    tc.tile_pool(name="ps1", bufs=2, space="PSUM") as ps1,
                ):
                    w_t = p1w.tile([128, KCH * PROJW], F32R, tag="w")
                    nc.sync.dma_start(
                        out=w_t[:].rearrange("p (kc q) -> p kc q", q=PROJW),
                        in_=wqkvT[:].rearrange("(kc p) q -> p kc q", p=128),
                    )
                    for tt in range(NTT):
                        xt = p1s.tile([128, KCH * 128], F32R, tag="xt")
                        nc.sync.dma_start(
                            out=xt[:].rearrange("p (kc t) -> p kc t", t=128),
                            in_=xT[:, tt * 128:(tt + 1) * 128].rearrange(
                                "(kc p) t -> p kc t", p=128),
                        )
                        q_ps = ps1.tile([128, QW], F32, tag="q")
                        kv_ps = ps1.tile([128, 2 * HD], F32, tag="kv")
                        for kc in range(KCH):
                            nc.tensor.matmul(
                                q_ps[:], xt[:, kc * 128:(kc + 1) * 128],
                                w_t[:, kc * PROJW: kc * PROJW + QW],
                                start=(kc == 0), stop=(kc == KCH - 1),
                            )
                            nc.tensor.matmul(
                                kv_ps[:], xt[:, kc * 128:(kc + 1) * 128],
                                w_t[:, kc * PROJW + QW: (kc + 1) * PROJW],
                                start=(kc == 0), stop=(kc == KCH - 1),
                            )
                        # RoPE (even/odd pre-separated into 64-wide halves)
                        ct = p1s.tile([128, 4 * 64], F32R, tag="cos")
                        st = p1s.tile([128, 4 * 64], F32R, tag="sin")
                        nc.sync.dma_start(out=ct[:], in_=cosn[tt * 128:(tt + 1) * 128, :])
                        nc.sync.dma_start(out=st[:], in_=sinn[tt * 128:(tt + 1) * 128, :])
                        rot = p1s.tile([128, QW + HD], F32R, tag="rot")
                        t1 = p1s.tile([128, 4 * 64], F32R, tag="t1")

                        qv = q_ps[:].rearrange("p (u hf) -> p u hf", hf=128)
                        qe, qo = qv[:, :, 0:64], qv[:, :, 64:128]
                        rv = rot[:, 0:QW].rearrange("p (u hf) -> p u hf", hf=128)
                        re, ro = rv[:, :, 0:64], rv[:, :, 64:128]
                        cv = ct[:].rearrange("p (u f) -> p u f", f=64)
                        sv = st[:].rearrange("p (u f) -> p u f", f=64)
                        tv = t1[:].rearrange("p (u f) -> p u f", f=64)
                        nc.vector.tensor_mul(re, qe, cv)
                        nc.vector.tensor_mul(tv, qo, sv)
                        nc.vector.tensor_sub(re, re, tv)
                        nc.vector.tensor_mul(ro, qe, sv)
                        nc.vector.tensor_mul(tv, qo, cv)
                        nc.vector.tensor_add(ro, ro, tv)
                        # k rope
                        ke, ko = kv_ps[:, 0:64], kv_ps[:, 64:128]
                        kre, kro = rot[:, QW:QW + 64], rot[:, QW + 64:QW + 128]
                        c1, s1, t1s = ct[:, 0:64], st[:, 0:64], t1[:, 0:64]
                        nc.vector.tensor_mul(kre, ke, c1)
                        nc.vector.tensor_mul(t1s, ko, s1)
                        nc.vector.tensor_sub(kre, kre, t1s)
                        nc.vector.tensor_mul(kro, ke, s1)
                        nc.vector.tensor_mul(t1s, ko, c1)
                        nc.vector.tensor_add(kro, kro, t1s)
                        # v copy (natural layout, chunk tt)
                        nc.scalar.copy(v_all[:, tt * 128:(tt + 1) * 128],
                                       kv_ps[:, 128:256])
                        # transposes: 4 q heads -> DRAM, 1 k -> resident K^T
                        for u in range(HL + 1):
                            tp_ps = ps1.tile([128, 128], F32R, tag="tp")
                            nc.tensor.transpose(
                                tp_ps[:], rot[:, u * 128:(u + 1) * 128], id_t[:])
                            if u < HL:
                                stg = p1s.tile([128, 128], F32R, tag="qstage")
                                nc.scalar.copy(stg[:], tp_ps[:])
                                nc.sync.dma_start(
                                    out=qT_d[u * 128:(u + 1) * 128,
                                             tt * 128:(tt + 1) * 128],
                                    in_=stg[:],
                                )
                            else:
                                nc.scalar.copy(
                                    kT_all[:, tt * 128:(tt + 1) * 128], tp_ps[:])

                # ------------- Phase 2: attention -> a2a_in ------------------
                with (
                    tc.tile_pool(name="p2s", bufs=3) as p2s,
                    tc.tile_pool(name="ps2", bufs=2, space="PSUM") as ps2,
                ):
                    for b in range(B):
                        for h in range(HL):
                            for tp in range(NTP):
                                j = b * NTP + tp          # token group 0..7
                                qt = p2s.tile([128, 512], F32R, tag="qt")
                                nc.sync.dma_start(
                                    out=qt[:],
                                    in_=qT_d[h * 128:(h + 1) * 128,
                                             b * T + tp * 512: b * T + (tp + 1) * 512],
                                )
                                ctx_ps = ps2.tile([128, 512], F32, tag="ctx")
                                sums_ps = ps2.tile([1, 512], F32, tag="sums")
                                for sc in range(NSC):
                                    g = (b * NSC + sc) * 128
                                    s_ps = ps2.tile([128, 512], F32, tag="s")
                                    nc.tensor.matmul(
                                        s_ps[:], kT_all[:, g:g + 128], qt[:],
                                        start=True, stop=True,
                                    )
                                    p_t = p2s.tile([128, 512], F32R, tag="p")
                                    nc.scalar.activation(
                                        p_t[:], s_ps[:], AF.Exp, scale=SCALE)
                                    nc.tensor.matmul(
                                        ctx_ps[:], v_all[:, g:g + 128], p_t[:],
                                        start=(sc == 0), stop=(sc == NSC - 1),
                                    )
                                    nc.tensor.matmul(
                                        sums_ps[:], oc_t[:], p_t[:],
                                        start=(sc == 0), stop=(sc == NSC - 1),
                                    )
                                recip = p2s.tile([1, 512], F32R, tag="recip")
                                nc.vector.reciprocal(recip[:], sums_ps[:])
                                bc_ps = ps2.tile([128, 512], F32, tag="s")
                                nc.tensor.matmul(bc_ps[:], or_t[:], recip[:],
                                                 start=True, stop=True)
                                ctx_sb = p2s.tile([128, 512], F32R, tag="ctxs")
                                nc.vector.tensor_copy(ctx_sb[:], ctx_ps[:])
                                nc.vector.tensor_mul(ctx_sb[:], ctx_sb[:], bc_ps[:])
                                nc.sync.dma_start(
                                    out=a2a_in[j * 512 + h * 128:
                                               j * 512 + (h + 1) * 128, :],
                                    in_=ctx_sb[:],
                                )

                # ------------- ctx exchange --------------------------------
                if collective:
                    nc.gpsimd.collective_compute(
                        "AllToAll",
                        mybir.AluOpType.bypass,
                        replica_groups=[list(range(N_CORES))],
                        ins=[a2a_in[:].opt()],
                        outs=[a2a_out[:].opt()],
                    )
                    ctx_src = a2a_out
                else:
                    ctx_src = a2a_in

                # ------------- Phase 3: o-proj (wo stationary, out^T) -------
                with (
                    tc.tile_pool(name="p3r", bufs=1) as p3r,
                    tc.tile_pool(name="p3s", bufs=3) as p3s,
                    tc.tile_pool(name="ps3", bufs=2, space="PSUM") as ps3,
                ):
                    ctxT_sb = p3r.tile([128, 32 * 512], F32R, tag="ctxT")
                    nc.sync.dma_start(
                        out=ctxT_sb[:].rearrange("p (cc t) -> p cc t", t=512),
                        in_=ctx_src[:].rearrange("(cc p) t -> p cc t", p=128),
                    )
                    for db in range(DIM // 128):
                        wo_tile = p3s.tile([128, 32 * 128], F32R, tag="wot")
                        nc.sync.dma_start(
                            out=wo_tile[:],
                            in_=woA[db * 128:(db + 1) * 128, :],
                        )
                        oT_ps = ps3.tile([128, 512], F32, tag="oT")
                        for cc in range(32):
                            nc.tensor.matmul(
                                oT_ps[:],
                                wo_tile[:, cc * 128:(cc + 1) * 128],
                                ctxT_sb[:, cc * 512:(cc + 1) * 512],
                                start=(cc == 0), stop=(cc == 31),
                            )
                        ost = p3s.tile([128, 512], F32, tag="ost")
                        nc.vector.tensor_copy(ost[:], oT_ps[:])
                        nc.sync.dma_start(
                            out=out_sl[db * 128:(db + 1) * 128, :],
                            in_=ost[:],
                        )
    nc.compile()
    return nc


def _rope_permutation():
    """Per-head permutation putting even dims first, odd dims second."""
    perm = np.empty(HD, dtype=np.int64)
    perm[:HD // 2] = np.arange(0, HD, 2)
    perm[HD // 2:] = np.arange(1, HD, 2)
    return perm


def _prep_inputs(x, wq, wk, wv, wo, freqs_cos, freqs_sin):
    x2d = np.ascontiguousarray(np.asarray(x, dtype=np.float32).reshape(TOK, DIM))
    xT = np.ascontiguousarray(x2d.T)
    wq = np.asarray(wq, dtype=np.float32)
    wk = np.asarray(wk, dtype=np.float32)
    wv = np.asarray(wv, dtype=np.float32)
    wo = np.asarray(wo, dtype=np.float32)
    fc = np.asarray(freqs_cos, dtype=np.float32)
    fs = np.asarray(freqs_sin, dtype=np.float32)

    perm = _rope_permutation()
    cosn = np.ascontiguousarray(np.tile(np.concatenate([fc, fc], axis=0), (1, 4)))
    sinn = np.ascontiguousarray(np.tile(np.concatenate([fs, fs], axis=0), (1, 4)))
    ones_col = np.ones((128, 1), np.float32)
    ones_row = np.ones((1, 128), np.float32)
    ident = np.eye(128, dtype=np.float32)

    # global core-major c' order: [core i][local head u][d] = head (i + 8u)
    idx = np.concatenate([
        np.arange(HD) + (i + N_KV_HEADS * u) * HD
        for i in range(N_CORES) for u in range(HL)])
    wo_r = np.ascontiguousarray(wo[:, idx].T)        # [c', D]
    woA = np.ascontiguousarray(
        wo_r.reshape(32, 128, 32, 128).transpose(2, 1, 0, 3).reshape(DIM, DIM))

    in_maps = []
    for c in range(N_CORES):
        # reference GQA (torch-style .repeat / jnp.tile): q-head g attends
        # kv-head g % 8, so core c owns q-heads {c, c+8, c+16, c+24} and
        # kv-head c.
        heads = [c + N_KV_HEADS * u for u in range(HL)]
        wq_c = wq.reshape(N_HEADS, HD, DIM)[heads][:, perm, :].reshape(QW, DIM)
        wk_c = wk[c * HD:(c + 1) * HD, :][perm, :]
        wv_c = wv[c * HD:(c + 1) * HD, :]
        wqkvT = np.ascontiguousarray(
            np.concatenate([wq_c, wk_c, wv_c], axis=0).T)
        in_maps.append({
            "xT": xT, "wqkvT": wqkvT, "woA": woA,
            "cosn": cosn, "sinn": sinn,
            "ones_col": ones_col, "ones_row": ones_row, "ident": ident,
        })
    return in_maps


def kernel(x, wq, wk, wv, wo, freqs_cos, freqs_sin,
           cache_k=None, cache_v=None, mask=None, start_pos=0, **_):
    assert int(start_pos) == 0, "kernel is specialized for start_pos=0"
    if "nc" not in _CACHE:
        _CACHE["nc"] = _build()
    nc = _CACHE["nc"]
    in_maps = _prep_inputs(x, wq, wk, wv, wo, freqs_cos, freqs_sin)
    res = bass_utils.run_bass_kernel_spmd(
        nc, in_maps, core_ids=list(range(N_CORES)))
    out = np.concatenate(
        [res.results[c]["out_sl"].T for c in range(N_CORES)], axis=0)
    return np.ascontiguousarray(out).reshape(B, T, DIM)



# revision 19
# speedup vs baseline: 108.9327x; 108.9327x over previous
"""Tensor-parallel GQA attention forward for Trainium2 (8 NeuronCores).

Sharding: tensor-parallel over heads.  Each core owns 4 q-heads and 1
kv-head (wq/wk/wv output-dim shard), processes the full 4096-token
stream, and a 2-way-split AllToAll redistributes ctx so core c ends
with output token rows [c*512, (c+1)*512).

v2 pipeline per core:
  1. Transposed QKV projection: q^T/k^T/v^T tiles come straight out of
     the PE (lhsT = w chunk, rhs = x^T chunk), so no Q/K transposes.
     RoPE on DVE in the [dims, tokens] layout (even/odd dim halves
     pre-separated by a host permutation of the weight rows).  Q^T is
     spilled to DRAM in bf16; K^T stays SBUF-resident in bf16; V^T is
     PE-transposed back to natural [s, d] bf16.
  2. Attention, h-outer loop.  Scores/PV matmuls in bf16 (N=512),
     exp on ScalarE over 2-bank PSUM tiles, softmax denominators via
     4-way column-tiled M=1 matmuls (concurrent on distinct col
     groups), PE-broadcast of the reciprocal, DVE normalize to bf16.
  3. ctx exchange in TWO bf16 AllToAlls: heads {0,1} fire at the
     attention midpoint (hidden behind heads {2,3}); heads {2,3} fire
     at the end (hidden behind o-proj half 1).
  4. o-proj split along the contraction: half 1 (a2a#1 data, 16 of 32
     cc chunks) accumulates into an SBUF f32 buffer while a2a#2 is in
     flight; half 2 adds the PSUM partials to the buffer and stores.
"""
import math
import numpy as np

import concourse.bacc as bacc
import concourse.mybir as mybir
import concourse.tile as tile
from concourse import bass_utils

F32R = mybir.dt.float32r
F32 = mybir.dt.float32
BF16 = mybir.dt.bfloat16
AF = mybir.ActivationFunctionType

N_CORES = 8
B, T, DIM = 2, 2048, 4096
N_HEADS, N_KV_HEADS, HD = 32, 8, 128
HL = N_HEADS // N_CORES            # 4 q heads per core
TOK = B * T                        # 4096
KCH = DIM // 128                   # 32 contraction chunks
QW = HL * HD                       # 512
PROJW = QW + 2 * HD                # 768 (q | k | v)
SCALE = 1.0 / math.sqrt(HD)
NP = TOK // 512                    # 8 token pieces
NPC = T // 256                     # 8 s-pairs per batch
OSL = TOK // N_CORES               # 512 output rows per core

_CACHE = {}


def _build(collective=True, reps=1):
    nc = bacc.Bacc("TRN2", target_bir_lowering=False, debug=False,
                   num_devices=N_CORES if collective else 1)
    xT = nc.dram_tensor("xT", [DIM, TOK], F32R, kind="ExternalInput")
    wqkvT = nc.dram_tensor("wqkvT", [DIM, PROJW], F32R, kind="ExternalInput")
    woA = nc.dram_tensor("woA", [DIM, DIM], BF16, kind="ExternalInput")
    cosT = nc.dram_tensor("cosT", [128, TOK], F32R, kind="ExternalInput")
    sinT = nc.dram_tensor("sinT", [128, TOK], F32R, kind="ExternalInput")
    ones_col = nc.dram_tensor("ones_col", [128, 1], BF16, kind="ExternalInput")
    ones_row = nc.dram_tensor("ones_row", [1, 128], F32R, kind="ExternalInput")
    identb = nc.dram_tensor("identb", [128, 128], BF16, kind="ExternalInput")
    out_sl = nc.dram_tensor("out_sl", [DIM, OSL], F32, kind="ExternalOutput")

    with tile.TileContext(nc) as tc:
        with (
            nc.allow_low_precision(reason="bf16 scores/ctx path"),
            tc.tile_pool(name="res", bufs=1) as res,
            tc.tile_pool(name="dram", bufs=1, space="DRAM") as dram,
        ):
            kT_all = res.tile([128, TOK], BF16, tag="kT")
            v_all = res.tile([128, TOK], BF16, tag="v")
            oc_t = res.tile([128, 1], BF16, tag="oc")
            or_t = res.tile([1, 128], F32R, tag="or")
            id_t = res.tile([128, 128], BF16, tag="id")
            nc.sync.dma_start(out=oc_t[:], in_=ones_col[:])
            nc.sync.dma_start(out=or_t[:], in_=ones_row[:])
            nc.sync.dma_start(out=id_t[:], in_=identb[:])

            qT_d = dram.tile([QW, TOK], BF16)
            a2a_in1 = dram.tile([TOK // 2, 512], BF16)
            a2a_in2 = dram.tile([TOK // 2, 512], BF16)
            a2a_out1 = dram.tile([TOK // 2, 512], BF16)
            a2a_out2 = dram.tile([TOK // 2, 512], BF16)

            for _rep in range(reps):
                # ---------- Phase 1: transposed projections + RoPE --------
                with (
                    tc.tile_pool(name="p1w", bufs=1) as p1w,
                    tc.tile_pool(name="p1x", bufs=2) as p1x,
                    tc.tile_pool(name="p1s", bufs=2) as p1s,
                    tc.tile_pool(name="ps1", bufs=1, space="PSUM") as ps1,
                    tc.tile_pool(name="ps1t", bufs=2, space="PSUM") as ps1t,
                ):
                    w_t = p1w.tile([128, KCH * PROJW], F32R, tag="w")
                    for kc in range(KCH):
                        # scalar DGE queue: keeps the bulk weight load off the
                        # sync queue so the first x tiles aren't blocked
                        nc.scalar.dma_start(
                            out=w_t[:, kc * PROJW:(kc + 1) * PROJW],
                            in_=wqkvT[kc * 128:(kc + 1) * 128, :],
                        )
                    for pi in range(NP):
                        t0 = pi * 512
                        # 6 accumulating PSUM tiles: E F G H (q pairs), K, V
                        ps_t = ps1.tile([128, 6 * 512], F32, tag="proj")
                        for g in range(4):          # 4 x-subloads of 8 kc
                            xt = p1x.tile([128, 8 * 512], F32R, tag="xt")
                            nc.sync.dma_start(
                                out=xt[:].rearrange("p (kc t) -> p kc t", t=512),
                                in_=xT[g * 1024:(g + 1) * 1024,
                                       t0:t0 + 512].rearrange(
                                    "(kc p) t -> p kc t", p=128),
                            )
                            for kl in range(8):
                                kc = g * 8 + kl
                                first = kc == 0
                                last = kc == KCH - 1
                                for u in range(6):
                                    nc.tensor.matmul(
                                        ps_t[:, u * 512:(u + 1) * 512],
                                        w_t[:, kc * PROJW + u * 128:
                                            kc * PROJW + (u + 1) * 128],
                                        xt[:, kl * 512:(kl + 1) * 512],
                                        start=first, stop=last,
                                    )
                        # free PSUM fast: copy to SBUF (ACT), then RoPE on DVE
                        efgh = p1s.tile([128, 4 * 512], F32R, tag="efgh")
                        ke_t = p1s.tile([64, 512], F32R, tag="ke")
                        ko_t = p1s.tile([64, 512], F32R, tag="ko")
                        vq = p1s.tile([128, 512], BF16, tag="vq")
                        for u in range(4):
                            nc.scalar.copy(efgh[:, u * 512:(u + 1) * 512],
                                           ps_t[:, u * 512:(u + 1) * 512])
                        nc.scalar.copy(ke_t[:], ps_t[0:64, 4 * 512:5 * 512])
                        nc.scalar.copy(ko_t[:], ps_t[64:128, 4 * 512:5 * 512])
                        nc.scalar.copy(vq[:], ps_t[:, 5 * 512:6 * 512])

                        ct = p1s.tile([128, 512], F32R, tag="cos")
                        st = p1s.tile([128, 512], F32R, tag="sin")
                        nc.sync.dma_start(out=ct[:], in_=cosT[:, t0:t0 + 512])
                        nc.sync.dma_start(out=st[:], in_=sinT[:, t0:t0 + 512])

                        tq = p1s.tile([128, 2 * 512], F32R, tag="tmp")
                        t1, t2 = tq[:, 0:512], tq[:, 512:1024]
                        # q RoPE on pairs (E,F), (G,H): E'=Ec-Fs, F'=Es+Fc
                        for pr in range(2):
                            e = efgh[:, (2 * pr) * 512:(2 * pr + 1) * 512]
                            f = efgh[:, (2 * pr + 1) * 512:(2 * pr + 2) * 512]
                            eo = p1s.tile([128, 512], BF16, tag=f"ro{pr}e")
                            fo = p1s.tile([128, 512], BF16, tag=f"ro{pr}o")
                            nc.vector.tensor_mul(t1, e, ct[:])
                            nc.vector.tensor_mul(t2, f, st[:])
                            nc.vector.tensor_sub(eo[:], t1, t2)
                            nc.vector.tensor_mul(t1, e, st[:])
                            nc.vector.tensor_mul(t2, f, ct[:])
                            nc.vector.tensor_add(fo[:], t1, t2)
                            nc.sync.dma_start(
                                out=qT_d[pr * 256:pr * 256 + 128, t0:t0 + 512],
                                in_=eo[:])
                            nc.sync.dma_start(
                                out=qT_d[pr * 256 + 128:pr * 256 + 256,
                                         t0:t0 + 512],
                                in_=fo[:])
                        # k RoPE on base-0 half tiles (SB-SB ops need equal
                        # base partitions; only the outputs straddle bases)
                        c64, s64 = ct[0:64, :], st[0:64, :]
                        th = p1s.tile([64, 2 * 512], F32R, tag="tmpk")
                        th1, th2 = th[:, 0:512], th[:, 512:1024]
                        kslice = kT_all[:, t0:t0 + 512]
                        nc.vector.tensor_mul(th1, ke_t[:], c64)
                        nc.vector.tensor_mul(th2, ko_t[:], s64)
                        nc.vector.tensor_sub(kslice[0:64, :], th1, th2)
                        nc.vector.tensor_mul(th1, ke_t[:], s64)
                        nc.vector.tensor_mul(th2, ko_t[:], c64)
                        nc.vector.tensor_add(kslice[64:128, :], th1, th2)
                        # v: transpose back to natural [s, d]
                        for j in range(4):
                            tp_ps = ps1t.tile([128, 128], BF16, tag="vt")
                            nc.tensor.transpose(
                                tp_ps[:], vq[:, j * 128:(j + 1) * 128], id_t[:])
                            nc.scalar.copy(
                                v_all[:, t0 + j * 128:t0 + (j + 1) * 128],
                                tp_ps[:])

                # ---------- Phase 2: attention (h-outer) + split a2a ------
                with (
                    tc.tile_pool(name="p2s", bufs=3) as p2s,
                    tc.tile_pool(name="p2p", bufs=4) as p2p,
                    tc.tile_pool(name="ps2s", bufs=3, space="PSUM") as ps2s,
                    tc.tile_pool(name="ps2", bufs=1, space="PSUM") as ps2,
                ):
                    for h in range(HL):
                        for b in range(B):
                            for tp in range(4):
                                t0 = b * T + tp * 512
                                qt = p2s.tile([128, 512], BF16, tag="qt")
                                eb = (h // 2) * 256 + (h % 2) * 64
                                ob = eb + 128
                                nc.sync.dma_start(
                                    out=qt[0:64, :],
                                    in_=qT_d[eb:eb + 64, t0:t0 + 512])
                                nc.sync.dma_start(
                                    out=qt[64:128, :],
                                    in_=qT_d[ob:ob + 64, t0:t0 + 512])
                                ctx_ps = ps2.tile([128, 512], F32, tag="ctx")
                                sums_ps = ps2.tile([128, 512], F32, tag="sums")
                                pts = []
                                for pc in range(NPC):
                                    s0 = b * T + pc * 256
                                    s_ps = ps2s.tile([128, 1024], F32, tag="s")
                                    for j in range(2):
                                        nc.tensor.matmul(
                                            s_ps[:, j * 512:(j + 1) * 512],
                                            kT_all[:, s0 + j * 128:
                                                   s0 + (j + 1) * 128],
                                            qt[:], start=True, stop=True)
                                    p_t = p2p.tile([128, 1024], BF16, tag="p")
                                    nc.scalar.activation(
                                        p_t[:], s_ps[:], AF.Exp, scale=SCALE)
                                    for j in range(2):
                                        nc.tensor.matmul(
                                            ctx_ps[:],
                                            v_all[:, s0 + j * 128:
                                                  s0 + (j + 1) * 128],
                                            p_t[:, j * 512:(j + 1) * 512],
                                            start=(pc == 0 and j == 0),
                                            stop=(pc == NPC - 1 and j == 1))
                                    pts.append(p_t)
                                    if pc % 2 == 1:
                                        g = pc // 2
                                        for j in range(4):
                                            pj = pts[j // 2]
                                            nc.tensor.matmul(
                                                sums_ps[32 * j:32 * j + 1, :],
                                                oc_t[:],
                                                pj[:, (j % 2) * 512:
                                                   (j % 2 + 1) * 512],
                                                start=(g == 0), stop=(g == 3),
                                                tile_position=(0, 32 * j))
                                        pts = []
                                # TensorTensor allows at most one PSUM input:
                                # copy the 4 col-tiled sum rows to SBUF first
                                s4 = p2s.tile([1, 4 * 512], F32R, tag="s4")
                                for j in range(4):
                                    nc.vector.tensor_copy(
                                        s4[:, j * 512:(j + 1) * 512],
                                        sums_ps[32 * j:32 * j + 1, :])
                                sA = p2s.tile([1, 512], F32R, tag="sA")
                                sB = p2s.tile([1, 512], F32R, tag="sB")
                                sC = p2s.tile([1, 512], F32R, tag="sC")
                                rc = p2s.tile([1, 512], F32R, tag="rc")
                                nc.vector.tensor_add(
                                    sA[:], s4[:, 0:512], s4[:, 512:1024])
                                nc.vector.tensor_add(
                                    sB[:], s4[:, 1024:1536], s4[:, 1536:2048])
                                nc.vector.tensor_add(sC[:], sA[:], sB[:])
                                nc.vector.reciprocal(rc[:], sC[:])
                                bc_ps = ps2.tile([128, 512], F32, tag="sums")
                                nc.tensor.matmul(bc_ps[:], or_t[:], rc[:],
                                                 start=True, stop=True)
                                bc_sb = p2s.tile([128, 512], F32R, tag="bcs")
                                nc.vector.tensor_copy(bc_sb[:], bc_ps[:])
                                ctxb = p2s.tile([128, 512], BF16, tag="ctxb")
                                nc.vector.tensor_mul(
                                    ctxb[:], ctx_ps[:], bc_sb[:])
                                dst = a2a_in1 if h < 2 else a2a_in2
                                r0 = (b * 4 + tp) * 256 + (h % 2) * 128
                                nc.sync.dma_start(
                                    out=dst[r0:r0 + 128, :], in_=ctxb[:])
                        if collective and h == 1:
                            nc.gpsimd.collective_compute(
                                "AllToAll", mybir.AluOpType.bypass,
                                replica_groups=[list(range(N_CORES))],
                                ins=[a2a_in1[:].opt()],
                                outs=[a2a_out1[:].opt()],
                            )
                        if collective and h == 3:
                            nc.gpsimd.collective_compute(
                                "AllToAll", mybir.AluOpType.bypass,
                                replica_groups=[list(range(N_CORES))],
                                ins=[a2a_in2[:].opt()],
                                outs=[a2a_out2[:].opt()],
                            )

                # ---------- Phase 3: o-proj, contraction split in 2 -------
                ctx1_src = a2a_out1 if collective else a2a_in1
                ctx2_src = a2a_out2 if collective else a2a_in2
                with (
                    tc.tile_pool(name="p3r", bufs=1) as p3r,
                    tc.tile_pool(name="p3s", bufs=3) as p3s,
                    tc.tile_pool(name="ps3", bufs=2, space="PSUM") as ps3,
                ):
                    acc = p3r.tile([128, KCH * 512], F32, tag="acc")
                    ctxT1 = p3r.tile([128, 16 * 512], BF16, tag="ctxT1")
                    # scalar DGE queue: the collective-dependent ctx loads
                    # must not head-of-line-block the wo_tile stream
                    nc.scalar.dma_start(
                        out=ctxT1[:].rearrange("p (cc t) -> p cc t", t=512),
                        in_=ctx1_src[:].rearrange("(cc p) t -> p cc t", p=128),
                    )
                    for db in range(DIM // 128):
                        wo_tile = p3s.tile([128, 16 * 128], BF16, tag="wot")
                        nc.sync.dma_start(
                            out=wo_tile[:],
                            in_=woA[db * 128:(db + 1) * 128, 0:2048],
                        )
                        oT_ps = ps3.tile([128, 512], F32, tag="oT")
                        for cc in range(16):
                            nc.tensor.matmul(
                                oT_ps[:],
                                wo_tile[:, cc * 128:(cc + 1) * 128],
                                ctxT1[:, cc * 512:(cc + 1) * 512],
                                start=(cc == 0), stop=(cc == 15),
                            )
                        nc.vector.tensor_copy(acc[:, db * 512:(db + 1) * 512],
                                              oT_ps[:])
                    ctxT2 = p3r.tile([128, 16 * 512], BF16, tag="ctxT2")
                    nc.scalar.dma_start(
                        out=ctxT2[:].rearrange("p (cc t) -> p cc t", t=512),
                        in_=ctx2_src[:].rearrange("(cc p) t -> p cc t", p=128),
                    )
                    for db in range(DIM // 128):
                        wo_tile = p3s.tile([128, 16 * 128], BF16, tag="wot")
                        nc.sync.dma_start(
                            out=wo_tile[:],
                            in_=woA[db * 128:(db + 1) * 128, 2048:4096],
                        )
                        oT_ps = ps3.tile([128, 512], F32, tag="oT")
                        for cc in range(16):
                            nc.tensor.matmul(
                                oT_ps[:],
                                wo_tile[:, cc * 128:(cc + 1) * 128],
                                ctxT2[:, cc * 512:(cc + 1) * 512],
                                start=(cc == 0), stop=(cc == 15),
                            )
                        ost = p3s.tile([128, 512], F32, tag="ost")
                        nc.vector.tensor_add(
                            ost[:], oT_ps[:], acc[:, db * 512:(db + 1) * 512])
                        nc.sync.dma_start(
                            out=out_sl[db * 128:(db + 1) * 128, :],
                            in_=ost[:],
                        )
    nc.compile()
    return nc


def _rope_permutation():
    """Per-head permutation putting even dims first, odd dims second."""
    perm = np.empty(HD, dtype=np.int64)
    perm[:HD // 2] = np.arange(0, HD, 2)
    perm[HD // 2:] = np.arange(1, HD, 2)
    return perm


def _prep_inputs(x, wq, wk, wv, wo, freqs_cos, freqs_sin):
    x2d = np.ascontiguousarray(np.asarray(x, dtype=np.float32).reshape(TOK, DIM))
    xT = np.ascontiguousarray(x2d.T)
    wq = np.asarray(wq, dtype=np.float32)
    wk = np.asarray(wk, dtype=np.float32)
    wv = np.asarray(wv, dtype=np.float32)
    wo = np.asarray(wo, dtype=np.float32)
    fc = np.asarray(freqs_cos, dtype=np.float32)
    fs = np.asarray(freqs_sin, dtype=np.float32)

    perm = _rope_permutation()
    # [128, TOK] tables: rows = freq index (0..63) twice, cols = token
    cosT = np.ascontiguousarray(
        np.tile(np.concatenate([fc, fc], axis=0).T, (2, 1)))
    sinT = np.ascontiguousarray(
        np.tile(np.concatenate([fs, fs], axis=0).T, (2, 1)))
    ones_col = to_bf16(np.ones((128, 1), np.float32))
    ones_row = np.ones((1, 128), np.float32)
    ident = to_bf16(np.eye(128, dtype=np.float32))

    # o-proj column order matches the split-a2a arrival order:
    # half 1: [src core i][local head u in {0,1}]; half 2: u in {2,3}
    idx = np.concatenate(
        [np.arange(HD) + (i + N_KV_HEADS * u) * HD
         for i in range(N_CORES) for u in range(2)]
        + [np.arange(HD) + (i + N_KV_HEADS * u) * HD
           for i in range(N_CORES) for u in range(2, 4)])
    wo_r = np.ascontiguousarray(wo[:, idx].T)        # [c', D]
    woA = np.ascontiguousarray(
        wo_r.reshape(32, 128, 32, 128).transpose(2, 1, 0, 3).reshape(DIM, DIM))
    woA = to_bf16(woA)

    in_maps = []
    for c in range(N_CORES):
        heads = [c + N_KV_HEADS * u for u in range(HL)]
        wq_h = wq.reshape(N_HEADS, HD, DIM)[heads]   # [4, HD, DIM]
        ev, od = perm[:64], perm[64:]
        # row blocks: E=[h0e h1e] F=[h0o h1o] G=[h2e h3e] H=[h2o h3o]
        rows = []
        for prb in range(2):
            h0, h1 = 2 * prb, 2 * prb + 1
            rows.append(np.concatenate([wq_h[h0][ev], wq_h[h1][ev]], axis=0))
            rows.append(np.concatenate([wq_h[h0][od], wq_h[h1][od]], axis=0))
        wk_c = wk[c * HD:(c + 1) * HD, :][perm, :]
        wv_c = wv[c * HD:(c + 1) * HD, :]
        wqkvT = np.ascontiguousarray(
            np.concatenate(rows + [wk_c, wv_c], axis=0).T)
        in_maps.append({
            "xT": xT, "wqkvT": wqkvT, "woA": woA,
            "cosT": cosT, "sinT": sinT,
            "ones_col": ones_col, "ones_row": ones_row, "identb": ident,
        })
    return in_maps


def to_bf16(a):
    import ml_dtypes
    return np.asarray(a, dtype=np.float32).astype(ml_dtypes.bfloat16)


def kernel(x, wq, wk, wv, wo, freqs_cos, freqs_sin,
           cache_k=None, cache_v=None, mask=None, start_pos=0, **_):
    assert int(start_pos) == 0, "kernel is specialized for start_pos=0"
    if "nc" not in _CACHE:
        _CACHE["nc"] = _build()
    nc = _CACHE["nc"]
    in_maps = _prep_inputs(x, wq, wk, wv, wo, freqs_cos, freqs_sin)
    res = bass_utils.run_bass_kernel_spmd(
        nc, in_maps, core_ids=list(range(N_CORES)))
    out = np.concatenate(
        [res.results[c]["out_sl"].T for c in range(N_CORES)], axis=0)
    return np.ascontiguousarray(out).reshape(B, T, DIM)
